# revision 1
# baseline (speedup 1.0000x reference)
"""GAT layer (nn_GAT_49589692400146) on 8 TRN2 NeuronCores.

Strategy (row-shard over nodes, SPMD — every core runs identical code):
  - Host: x.T (fp16) with a per-core column permutation that puts the core's
    own 768 node columns first; mask = adj-shard transposed to [j, i] layout
    (fp16 {0,1}), same permutation on j.
  - Device, per core:
      P0: Waug = [W | W@a1 | W@a2] (fp16), z1 row for local nodes,
          broadcast to zb1 [128, 768].
      P1: for each 128-node chunk: [Wh | z1 | z2] = xT_chunk.T @ Waug via
          fp16 matmuls; Wh chunk -> SBUF fp16 (+ ones column), z2 -> column.
      P2: e^T tiles [j, i]: t = zb1 + z2[j] (DVE), t = Prelu(t, 0.2) (ACT),
          p = exp(t - 8) fp16 (ACT), p *= mask (DVE);
          accumulate [numer | den] = p^T @ [Wh | 1] into 6 PSUM banks (PE).
          (softmax denominator = appended ones column; no max-subtraction
          needed: logits are bounded, the -8 shift keeps exp in fp16 range
          and cancels in the normalization.)
      P3: h = numer/den, he = elu(h) = min(exp(h)-1, relu(h)),
          hc[i] = he . fcW_top  (DVE), s_c = column-sum(he) (PE ones-matmul).
  - Host: out = concat(hc) + (sum_c s_c) @ fcW_bot + fcb.
"""

import os
import numpy as np

import concourse.bacc as bacc
import concourse.tile as tile
import concourse.mybir as mybir
from concourse import bass_utils

F32 = mybir.dt.float32
F16 = mybir.dt.float16
ALU = mybir.AluOpType
AF = mybir.ActivationFunctionType

NCORES = 8
N_FULL = 6144
NF = 512
NH = 256
ALPHA = 0.2
EXP_SHIFT = 8.0

_BUILD_CACHE = {}


def _build(NN, R):
    """Build the per-core SPMD module. NN = total nodes (j dim), R = local rows."""
    PHASES = os.environ.get("GAT_PHASES", "123")
    P = 128
    T = NN // P          # j-tiles / node chunks
    IC = R // P          # i-chunks
    KT = NF // P         # 4 k-tiles over features
    KH = NH // P         # 2 k-tiles over hidden
    GS = min(8, T)       # j-tiles per group
    NG = T // GS
    assert T % GS == 0 and R % P == 0 and NN % P == 0
    group_sizes = [GS] * NG
    group_starts = [sum(group_sizes[:i]) for i in range(len(group_sizes))]

    nc = bacc.Bacc("TRN2", target_bir_lowering=False, debug=False)

    xTp = nc.dram_tensor("xTp", [NF, NN], F16, kind="ExternalInput").ap()
    maskp = nc.dram_tensor("maskp", [NN, R], F16, kind="ExternalInput").ap()
    w_in = nc.dram_tensor("w_in", [NF, NH], F16, kind="ExternalInput").ap()
    wt_in = nc.dram_tensor("wt_in", [NH, NF], F16, kind="ExternalInput").ap()
    a_in = nc.dram_tensor("a_in", [P, 2 * KH], F16, kind="ExternalInput").ap()
    fcw_in = nc.dram_tensor("fcw_in", [1, NH], F16, kind="ExternalInput").ap()

    hc_out = nc.dram_tensor("hc_out", [R, 1], F32, kind="ExternalOutput").ap()
    sc_out = nc.dram_tensor("sc_out", [1, NH], F32, kind="ExternalOutput").ap()

    xTp_r = xTp.rearrange("(k p) n -> k p n", p=P)      # [KT, 128, NN]
    maskp_r = maskp.rearrange("(t p) r -> t p r", p=P)  # [T, 128, R]
    w_r = w_in.rearrange("(k p) h -> k p h", p=P)       # [KT, 128, NH]
    wt_r = wt_in.rearrange("(k p) f -> k p f", p=P)     # [KH, 128, NF]

    with tile.TileContext(nc) as tc:
        import contextlib

        with contextlib.ExitStack() as ctx:
            pXT = ctx.enter_context(tc.tile_pool(name="pXT", bufs=1))
            pCst = ctx.enter_context(tc.tile_pool(name="pCst", bufs=1))
            pWho = ctx.enter_context(tc.tile_pool(name="pWho", bufs=1))
            pT = ctx.enter_context(tc.tile_pool(name="pT", bufs=2))
            pP = ctx.enter_context(tc.tile_pool(name="pP", bufs=3))
            pM = ctx.enter_context(tc.tile_pool(name="pM", bufs=3))
            pS = ctx.enter_context(tc.tile_pool(name="pS", bufs=6))
            pDram = ctx.enter_context(tc.tile_pool(name="pDram", bufs=1, space="DRAM"))
            psW = ctx.enter_context(tc.tile_pool(name="psW", bufs=(2 if "2" in PHASES else 6), space="PSUM"))
            psA = ctx.enter_context(tc.tile_pool(name="psA", bufs=1, space="PSUM"))

            # ---- constants / weights in (small, critical DMAs first) ----

            waug = []
            wa1c = []
            for k in range(KT):
                wk = pCst.tile([P, NH + 1], F16, tag=f"waug{k}")
                nc.sync.dma_start(wk[:, 1:NH + 1], w_r[k])
                waug.append(wk)
                w1 = pCst.tile([P, 1], F16, tag=f"wa1c{k}")
                wa1c.append(w1)

            wt_sb = []
            for k in range(KH):
                wtk = pCst.tile([P, NF], F16, tag=f"wt{k}")
                nc.sync.dma_start(wtk[:], wt_r[k])
                wt_sb.append(wtk)

            a_sb = pCst.tile([P, 2 * KH], F16, tag="a_sb")
            nc.sync.dma_start(a_sb[:], a_in)

            fcwb = pCst.tile([P, NH], F16, tag="fcwb")
            nc.gpsimd.dma_start(fcwb[:], fcw_in.partition_broadcast(P))

            ones_col = pCst.tile([P, 1], F16, tag="ones_col")
            nc.gpsimd.memset(ones_col[:], 1.0)
            shift_col = pCst.tile([P, 1], F32, tag="shift_col")
            nc.gpsimd.memset(shift_col[:], -EXP_SHIFT)

            xt = []
            for k in range(KT):
                xk = pXT.tile([P, NN], F16, tag=f"xt{k}")
                xt.append(xk)
            if NN % 768 == 0:
                pieces = [(p0, 768) for p0 in range(0, NN, 768)]
            else:
                pieces = [(0, NN)]
            for p0, ln in pieces:
                for k in range(KT):
                    nc.sync.dma_start(
                        xt[k][:, p0:p0 + ln], xTp_r[k][:, p0:p0 + ln]
                    )

            # ---- P0: [Wa2 | Wa1] per feat chunk; Wa2 -> waug col 0, Wa1 -> wa1c ----
            for mc in range(KT):
                pwa = psW.tile([P, 2], F32, tag="work")
                for k in range(KH):
                    nc.tensor.matmul(
                        pwa[:],
                        wt_sb[k][:, mc * P:(mc + 1) * P],
                        a_sb[:, 2 * k:2 * k + 2],
                        start=(k == 0),
                        stop=(k == KH - 1),
                    )
                nc.vector.tensor_copy(waug[mc][:, 0:1], pwa[:, 0:1])
                nc.vector.tensor_copy(wa1c[mc][:], pwa[:, 1:2])

            # z1 row for local nodes: z1 = x_local @ (W @ a1) -> [1, R], in halves
            z1row = pCst.tile([1, R], F16, tag="z1row")
            HR = R // 2
            for h in range(2):
                z1p = psW.tile([1, HR], F32, tag="work", name=f"z1p{h}")
                for k in range(KT):
                    nc.tensor.matmul(
                        z1p[:],
                        wa1c[k][:],
                        xt[k][:, h * HR:(h + 1) * HR],
                        start=(k == 0),
                        stop=(k == KT - 1),
                    )
                nc.vector.tensor_copy(z1row[0:1, h * HR:(h + 1) * HR], z1p[:])
            zscr = pDram.tile([1, R], F16, tag="zscr")
            nc.gpsimd.dma_start(zscr[:], z1row[:])
            zb1 = pCst.tile([P, R], F16, tag="zb1")
            nc.gpsimd.dma_start(zb1[:], zscr[0:1, :].partition_broadcast(P))

            # ---- P1: per chunk [Wh | z1 | z2]; Wh -> fp16 SBUF (+ones), z2 col ----
            z2g = [
                pCst.tile([P, gs], F32, tag=f"z2g{g}", name=f"z2g{g}")
                for g, gs in enumerate(group_sizes)
            ]
            who = []
            for t in range(T):
                wo = pWho.tile([P, NH + 2], F16, tag=f"who{t}")
                nc.gpsimd.memset(wo[:, NH + 1:NH + 2], 1.0)
                who.append(wo)

            # ---- P2 accumulators ----
            acc = [
                psA.tile([P, NH + 1], F32, tag=f"acc{i}", name=f"acc{i}")
                for i in range(IC if "2" in PHASES else 0)
            ]

            # ---- P1 chunks and P2 groups interleaved in trace order ----
            for g, gs in enumerate(group_sizes):
                g0 = group_starts[g]
                mk = None
                if "2" in PHASES:
                    mk = pM.tile([P, GS * R], F16, tag="mask", name=f"mk{g}")
                    for t in range(gs):
                        jt = g0 + t
                        nc.sync.dma_start(mk[:, t * R:(t + 1) * R], maskp_r[jt])

                for t in range(gs):
                    jt = g0 + t
                    pc = psW.tile([P, NH + 1], F32, tag="work", name=f"pc{jt}")
                    for k in range(KT):
                        nc.tensor.matmul(
                            pc[:],
                            xt[k][:, jt * P:(jt + 1) * P],
                            waug[k][:],
                            start=(k == 0),
                            stop=(k == KT - 1),
                        )
                    nc.vector.tensor_copy(who[jt][:, 0:NH + 1], pc[:])
                    nc.vector.tensor_copy(z2g[g][:, t:t + 1], pc[:, 0:1])

                if "2" not in PHASES:
                    continue
                W2 = gs * R
                tm = pT.tile([P, GS * R], F16, tag="tmega", name=f"tm{g}")
                for t in range(gs):
                    jt = g0 + t
                    nc.vector.tensor_scalar_add(
                        tm[:, t * R:(t + 1) * R], zb1[:], z2g[g][:, t:t + 1]
                    )
                pm = pP.tile([P, GS * R], F16, tag="pmega", name=f"pm{g}")
                if g == 0:
                    for h0, h1 in [(0, W2 // 2), (W2 // 2, W2)]:
                        nc.scalar.activation(
                            tm[:, h0:h1], tm[:, h0:h1], AF.Prelu, alpha=ALPHA
                        )
                        for q0 in range(h0, h1, W2 // 4):
                            sl = slice(q0, q0 + W2 // 4)
                            nc.scalar.activation(pm[:, sl], tm[:, sl], AF.Exp, bias=shift_col[:])
                            nc.vector.tensor_tensor(pm[:, sl], pm[:, sl], mk[:, sl], op=ALU.mult)
                else:
                    nc.scalar.activation(tm[:], tm[:], AF.Prelu, alpha=ALPHA)
                    Q = W2 // 4
                    for h0 in range(0, W2, Q):
                        sl = slice(h0, h0 + Q)
                        nc.scalar.activation(pm[:, sl], tm[:, sl], AF.Exp, bias=shift_col[:])
                        nc.vector.tensor_tensor(pm[:, sl], pm[:, sl], mk[:, sl], op=ALU.mult)

                last_g = g == len(group_sizes) - 1
                if last_g:
                    for i in range(IC):
                        for t in range(gs):
                            jt = g0 + t
                            nc.tensor.matmul(
                                acc[i][:],
                                pm[:, t * R + i * P:t * R + (i + 1) * P],
                                who[jt][:, 1:NH + 2],
                                start=(g == 0 and t == 0),
                                stop=(t == gs - 1),
                            )
                else:
                    for t in range(gs):
                        jt = g0 + t
                        for i in range(IC):
                            nc.tensor.matmul(
                                acc[i][:],
                                pm[:, t * R + i * P:t * R + (i + 1) * P],
                                who[jt][:, 1:NH + 2],
                                start=(g == 0 and t == 0),
                                stop=False,
                            )

            # ---- P3: normalize, ELU, outputs ----
            hc_sb = pCst.tile([P, IC], F32, tag="hc_sb")
            nc.gpsimd.memset(hc_sb[:], 0.0)
            sacc = psW.tile([1, NH], F32, tag="work")
            s_sb = pCst.tile([1, NH], F32, tag="s_sb")
            nc.gpsimd.memset(s_sb[:], 0.0)
            for i in range(IC if ("3" in PHASES and "2" in PHASES) else 0):
                rec = pS.tile([P, 1], F32, tag="rec")
                nc.vector.reciprocal(rec[:], acc[i][:, NH:NH + 1])
                h = pS.tile([P, NH], F32, tag="h")
                nc.vector.tensor_scalar_mul(h[:], acc[i][:, 0:NH], rec[:])
                ex = pS.tile([P, NH], F32, tag="ex")
                nc.scalar.activation(ex[:], h[:], AF.Exp)
                rl = pS.tile([P, NH], F32, tag="rl")
                nc.vector.tensor_scalar_max(rl[:], h[:], 0.0)
                he = pS.tile([P, NH], F16, tag="he")
                nc.vector.scalar_tensor_tensor(
                    he[:], ex[:], -1.0, rl[:], ALU.add, ALU.min
                )
                nc.tensor.matmul(
                    sacc[:], ones_col[:], he[:],
                    start=(i == 0), stop=(i == IC - 1),
                )
                hw = pS.tile([P, NH], F16, tag="hw")
                nc.vector.scalar_tensor_tensor(
                    he[:] if False else hw[:], he[:], 1.0, fcwb[:],
                    ALU.mult, ALU.mult, accum_out=hc_sb[:, i:i + 1]
                )

            if "3" in PHASES and "2" in PHASES:
                nc.vector.tensor_copy(s_sb[:], sacc[:])
            nc.sync.dma_start(sc_out, s_sb[:])
            nc.sync.dma_start(
                hc_out.rearrange("(a p) o -> p (a o)", p=P), hc_sb[:]
            )

    nc.compile()
    return nc


def _get_module(NN, R):
    key = (NN, R, os.environ.get("GAT_PHASES", "123"))
    if key not in _BUILD_CACHE:
        _BUILD_CACHE[key] = _build(NN, R)
    return _BUILD_CACHE[key]


def _make_in_maps(x, adj, W, a, fcW, n_cores=NCORES):
    NN = x.shape[0]
    R = NN // n_cores
    P = 128
    KH = NH // P

    xT = np.ascontiguousarray(x.T).astype(np.float16)        # [NF, NN]
    W16 = W.astype(np.float16)
    WT16 = np.ascontiguousarray(W16.T)                       # [NH, NF]
    a16 = a.astype(np.float16)[:, 0]
    a_t = np.zeros((P, 2 * KH), np.float16)
    for k in range(KH):
        a_t[:, 2 * k] = a16[NH + k * P:NH + (k + 1) * P]      # a2 chunk k
        a_t[:, 2 * k + 1] = a16[k * P:(k + 1) * P]            # a1 chunk k
    fcw_row = fcW[:NH, 0].astype(np.float16)[None, :]        # [1, NH]

    maskT = (adj > 0).astype(np.float16).T                   # [NN (j), NN (i)]

    in_maps = []
    for c in range(n_cores):
        r0, r1 = c * R, (c + 1) * R
        xTp = np.concatenate([xT[:, r0:r1], xT[:, :r0], xT[:, r1:]], axis=1)
        mT = maskT[:, r0:r1]                                  # [NN, R]
        maskp = np.concatenate([mT[r0:r1], mT[:r0], mT[r1:]], axis=0)
        in_maps.append({
            "xTp": np.ascontiguousarray(xTp),
            "maskp": np.ascontiguousarray(maskp),
            "w_in": W16,
            "wt_in": WT16,
            "a_in": a_t,
            "fcw_in": fcw_row,
        })
    return in_maps


def _run_sharded(x, adj, W, a, fcW, fcb, n_cores=NCORES, **run_kwargs):
    NN = x.shape[0]
    R = NN // n_cores
    nc = _get_module(NN, R)
    in_maps = _make_in_maps(x, adj, W, a, fcW, n_cores)

    res = bass_utils.run_bass_kernel_spmd(
        nc, in_maps, core_ids=list(range(n_cores)), **run_kwargs
    )

    hc = np.concatenate([res.results[c]["hc_out"] for c in range(n_cores)], axis=0)
    s = np.sum([res.results[c]["sc_out"] for c in range(n_cores)], axis=0)[0]  # [NH]
    const = s.astype(np.float64) @ fcW[NH:, 0].astype(np.float64) + float(fcb[0])
    out = hc + np.float32(const)
    return out.astype(np.float32), res


def kernel(x, adj, W, a, fcW, fcb):
    out, _ = _run_sharded(
        np.asarray(x), np.asarray(adj), np.asarray(W),
        np.asarray(a), np.asarray(fcW), np.asarray(fcb),
    )
    return out



# revision 3
# speedup vs baseline: 1.3163x; 1.3163x over previous
"""GAT layer (nn_GAT_49589692400146) on 8 TRN2 NeuronCores.

Strategy (row-shard over nodes i, SPMD; no cross-core comm):

Math reformulation that removes all full-matrix transcendentals:
  e_ij = lrelu(z1_i + z2_j),  z1 = x@(W a1), z2 = x@(W a2)  (1-D, host fp32)
  p_ij = m_ij * exp(lrelu(t) - 8)
       = m_ij * max(exp(t - 8), exp(0.2 t - 8))          (exp monotone)
       = [m_ij * u1_i] * max(u2_j, (v1_i/u1_i) * v2_j)   (separable!)
  with u = exp(z - 4), v = exp(0.2 z - 4).
  Host precomputes MU[j,i] = m_ij * u1_i (the mask DMA payload, fp16),
  RB[i] = exp(-0.8 z1_i), and per-j columns u2/v2 (fp32).

Device, per core (768 local i-columns, all 6144 j rows):
  P1: Wh chunks [128 nodes, 256] = xT_chunk.T @ W (fp16 matmuls, PSUM),
      evicted to fp16 SBUF by the ACT engine (Copy), + ones column -> who.
  G:  G = (RB_bcast * v2_j) max u2_j     one 4x tensor_scalar per j-tile.
  p:  p = MU * G                          one 2x tensor_tensor per group.
  P2: acc[i] += p_tile^T @ [Wh | 1]       PE, PSUM accumulate [numer|den].
  P3: h = numer/den, he = elu(h) = min(exp(h)-1, relu(h)),
      hc_i = he . fcW_top (DVE accum), s = column-sum(he) (ones matmul).
Host: out = concat(hc) + (sum_c s_c) @ fcW_bot + fcb.
"""

import numpy as np

import concourse.bacc as bacc
import concourse.tile as tile
import concourse.mybir as mybir
from concourse import bass_utils

F32 = mybir.dt.float32
F16 = mybir.dt.float16
ALU = mybir.AluOpType
AF = mybir.ActivationFunctionType

NCORES = 8
N_FULL = 6144
NF = 512
NH = 256
ALPHA = 0.2
ESH = 4.0  # per-factor exp shift; total 8 like the reference-safe baseline

_BUILD_CACHE = {}


def _group_sizes(T):
    """Small leading groups so PE starts early; sum == T."""
    sizes = []
    for s in (2, 4):
        if sum(sizes) + s <= T:
            sizes.append(s)
    while T - sum(sizes) >= 8:
        sizes.append(8)
    r = T - sum(sizes)
    if r:
        sizes.append(r)
    return sizes


def _build(NN, R):
    """Build the per-core SPMD module. NN = total nodes (j), R = local i."""
    P = 128
    T = NN // P
    IC = R // P
    KT = NF // P
    assert R % P == 0 and NN % P == 0
    group_sizes = _group_sizes(T)
    group_starts = [sum(group_sizes[:i]) for i in range(len(group_sizes))]
    NG = len(group_sizes)
    GSMAX = max(group_sizes)

    nc = bacc.Bacc("TRN2", target_bir_lowering=False, debug=False)

    w_in = nc.dram_tensor("w_in", [P, KT * NH], F16, kind="ExternalInput").ap()
    xTp = nc.dram_tensor("xTp", [NF, NN], F16, kind="ExternalInput").ap()
    mu_in = nc.dram_tensor("mu_in", [P, T * R], F16, kind="ExternalInput").ap()
    rb_in = nc.dram_tensor("rb_in", [1, R], F16, kind="ExternalInput").ap()
    uv_in = nc.dram_tensor("uv_in", [P, 2 * T], F32, kind="ExternalInput").ap()
    fcw_in = nc.dram_tensor("fcw_in", [1, NH], F16, kind="ExternalInput").ap()

    hc_out = nc.dram_tensor("hc_out", [R, 1], F32, kind="ExternalOutput").ap()
    sc_out = nc.dram_tensor("sc_out", [1, NH], F32, kind="ExternalOutput").ap()

    xTp_r = xTp.rearrange("(k p) n -> k p n", p=P)  # [KT, 128, NN]

    with tile.TileContext(nc) as tc:
        import contextlib

        with contextlib.ExitStack() as ctx:
            pXT = ctx.enter_context(tc.tile_pool(name="pXT", bufs=1))
            pCst = ctx.enter_context(tc.tile_pool(name="pCst", bufs=1))
            pWho = ctx.enter_context(tc.tile_pool(name="pWho", bufs=1))
            pMU = ctx.enter_context(tc.tile_pool(name="pMU", bufs=2))
            pG = ctx.enter_context(tc.tile_pool(name="pG", bufs=2))
            pP = ctx.enter_context(tc.tile_pool(name="pP", bufs=2))
            pS = ctx.enter_context(tc.tile_pool(name="pS", bufs=6))
            psW = ctx.enter_context(tc.tile_pool(name="psW", bufs=2, space="PSUM"))
            psA = ctx.enter_context(tc.tile_pool(name="psA", bufs=1, space="PSUM"))

            # ---- DMA queue: weights, then per-group xt pieces + MU ----
            wch = pCst.tile([P, KT * NH], F16, tag="wch")
            nc.sync.dma_start(wch[:], w_in)

            xt = [
                pXT.tile([P, NN], F16, tag=f"xt{k}", name=f"xt{k}")
                for k in range(KT)
            ]
            for g, gs in enumerate(group_sizes):
                c0, c1 = group_starts[g] * P, (group_starts[g] + gs) * P
                for k in range(KT):
                    nc.sync.dma_start(xt[k][:, c0:c1], xTp_r[k][:, c0:c1])
                if g == 0:
                    uv = pCst.tile([P, 2 * T], F32, tag="uv")
                    nc.sync.dma_start(uv[:], uv_in)
                    rbb = pCst.tile([P, R], F16, tag="rbb")
                    nc.gpsimd.dma_start(rbb[:], rb_in.partition_broadcast(P))
                    fcwb = pCst.tile([P, NH], F16, tag="fcwb")
                    nc.gpsimd.dma_start(fcwb[:], fcw_in.partition_broadcast(P))

            mug = []
            for g, gs in enumerate(group_sizes):
                m = pMU.tile([P, gs * R], F16, tag="mu", name=f"mu{g}")
                nc.sync.dma_start(
                    m[:], mu_in[:, group_starts[g] * R:(group_starts[g] + gs) * R]
                )
                mug.append(m)

            # ---- constants ----
            who = pWho.tile([P, T * (NH + 1)], F16, tag="who")
            who_r = who[:].rearrange("p (t c) -> p t c", c=NH + 1)
            nc.gpsimd.memset(who_r[:, :, NH:NH + 1], 1.0)

            ones_col = pCst.tile([P, 1], F16, tag="ones_col")
            nc.gpsimd.memset(ones_col[:], 1.0)
            hc_sb = pCst.tile([P, IC], F32, tag="hc_sb")
            nc.gpsimd.memset(hc_sb[:], 0.0)

            # ---- P2 accumulators ----
            acc = [
                psA.tile([P, NH + 1], F32, tag=f"acc{i}", name=f"acc{i}")
                for i in range(IC)
            ]

            for g, gs in enumerate(group_sizes):
                g0 = group_starts[g]

                # P1: Wh chunks of this group -> who (ACT evict)
                for t in range(gs):
                    jt = g0 + t
                    pc = psW.tile([P, NH], F32, tag="work", name=f"pc{jt}")
                    for k in range(KT):
                        nc.tensor.matmul(
                            pc[:],
                            xt[k][:, jt * P:(jt + 1) * P],
                            wch[:, k * NH:(k + 1) * NH],
                            start=(k == 0),
                            stop=(k == KT - 1),
                        )
                    nc.scalar.activation(
                        who[:, jt * (NH + 1):jt * (NH + 1) + NH], pc[:], AF.Copy
                    )

                # G tiles then p = MU*G for the whole group
                gg = pG.tile([P, gs * R], F16, tag="gg", name=f"gg{g}")
                for t in range(gs):
                    jt = g0 + t
                    nc.vector.tensor_scalar(
                        gg[:, t * R:(t + 1) * R],
                        rbb[:],
                        uv[:, 2 * jt:2 * jt + 1],
                        uv[:, 2 * jt + 1:2 * jt + 2],
                        op0=ALU.mult,
                        op1=ALU.max,
                    )
                pg = pP.tile([P, gs * R], F16, tag="pg", name=f"pg{g}")
                nc.vector.tensor_tensor(pg[:], mug[g][:], gg[:], op=ALU.mult)

                # P2: accumulate [numer | den]
                last_g = g == NG - 1
                if last_g:
                    for i in range(IC):
                        for t in range(gs):
                            jt = g0 + t
                            nc.tensor.matmul(
                                acc[i][:],
                                pg[:, t * R + i * P:t * R + (i + 1) * P],
                                who[:, jt * (NH + 1):(jt + 1) * (NH + 1)],
                                start=(g == 0 and t == 0),
                                stop=(t == gs - 1),
                            )
                else:
                    for t in range(gs):
                        jt = g0 + t
                        for i in range(IC):
                            nc.tensor.matmul(
                                acc[i][:],
                                pg[:, t * R + i * P:t * R + (i + 1) * P],
                                who[:, jt * (NH + 1):(jt + 1) * (NH + 1)],
                                start=(g == 0 and t == 0),
                                stop=False,
                            )

            # ---- P3: normalize, ELU, outputs ----
            sacc = psW.tile([1, NH], F32, tag="work")
            s_sb = pCst.tile([1, NH], F32, tag="s_sb")
            for i in range(IC):
                rec = pS.tile([P, 1], F32, tag="rec")
                nc.vector.reciprocal(rec[:], acc[i][:, NH:NH + 1])
                h = pS.tile([P, NH], F32, tag="h")
                nc.vector.tensor_scalar_mul(h[:], acc[i][:, 0:NH], rec[:])
                ex = pS.tile([P, NH], F32, tag="ex")
                nc.scalar.activation(ex[:], h[:], AF.Exp)
                rl = pS.tile([P, NH], F32, tag="rl")
                nc.scalar.activation(rl[:], h[:], AF.Relu)
                he = pS.tile([P, NH], F16, tag="he")
                nc.vector.scalar_tensor_tensor(
                    he[:], ex[:], -1.0, rl[:], ALU.add, ALU.min
                )
                nc.tensor.matmul(
                    sacc[:], ones_col[:], he[:],
                    start=(i == 0), stop=(i == IC - 1),
                )
                hw = pS.tile([P, NH], F16, tag="hw")
                nc.vector.scalar_tensor_tensor(
                    hw[:], he[:], 1.0, fcwb[:],
                    ALU.mult, ALU.mult, accum_out=hc_sb[:, i:i + 1]
                )

            nc.vector.tensor_copy(s_sb[:], sacc[:])
            nc.sync.dma_start(sc_out, s_sb[:])
            nc.sync.dma_start(
                hc_out.rearrange("(a p) o -> p (a o)", p=P), hc_sb[:]
            )

    nc.compile()
    return nc


def _get_module(NN, R):
    key = (NN, R)
    if key not in _BUILD_CACHE:
        _BUILD_CACHE[key] = _build(NN, R)
    return _BUILD_CACHE[key]


def _make_in_maps(x, adj, W, a, fcW, n_cores=NCORES):
    NN = x.shape[0]
    R = NN // n_cores
    P = 128
    T = NN // P
    KT = NF // P

    x64 = x.astype(np.float64)
    W64 = W.astype(np.float64)
    a64 = a.astype(np.float64)[:, 0]
    z1 = x64 @ (W64 @ a64[:NH])            # [NN]
    z2 = x64 @ (W64 @ a64[NH:])            # [NN]

    u1 = np.exp(z1 - ESH)
    u2 = np.exp(z2 - ESH)
    v1 = np.exp(ALPHA * z1 - ESH)
    v2 = np.exp(ALPHA * z2 - ESH)
    rb_full = (v1 / u1).astype(np.float16)  # exp(-0.8 z1)

    # xT (feature-major), packed W chunks
    xT = np.ascontiguousarray(x.T).astype(np.float16)         # [NF, NN]
    w_t = np.zeros((P, KT * NH), np.float16)
    for k in range(KT):
        w_t[:, k * NH:(k + 1) * NH] = W[k * P:(k + 1) * P, :].astype(np.float16)

    uv = np.zeros((P, 2 * T), np.float32)
    for t in range(T):
        uv[:, 2 * t] = v2[t * P:(t + 1) * P]
        uv[:, 2 * t + 1] = u2[t * P:(t + 1) * P]

    fcw_row = fcW[:NH, 0].astype(np.float16)[None, :]         # [1, NH]

    mask = (adj > 0)                                          # [i, j]
    in_maps = []
    for c in range(n_cores):
        r0, r1 = c * R, (c + 1) * R
        # MU[j, i_local] = mask[i, j] * u1[i]
        mu = (mask[r0:r1, :].T * u1[r0:r1][None, :]).astype(np.float16)  # [NN, R]
        # partition-major layout [p, t*R + i]
        mu_pm = np.ascontiguousarray(
            mu.reshape(T, P, R).transpose(1, 0, 2).reshape(P, T * R)
        )
        in_maps.append({
            "w_in": w_t,
            "xTp": xT,
            "mu_in": mu_pm,
            "rb_in": np.ascontiguousarray(rb_full[r0:r1][None, :]),
            "uv_in": uv,
            "fcw_in": fcw_row,
        })
    return in_maps


def _run_sharded(x, adj, W, a, fcW, fcb, n_cores=NCORES, **run_kwargs):
    NN = x.shape[0]
    R = NN // n_cores
    nc = _get_module(NN, R)
    in_maps = _make_in_maps(x, adj, W, a, fcW, n_cores)

    res = bass_utils.run_bass_kernel_spmd(
        nc, in_maps, core_ids=list(range(n_cores)), **run_kwargs
    )

    hc = np.concatenate([res.results[c]["hc_out"] for c in range(n_cores)], axis=0)
    s = np.sum([res.results[c]["sc_out"] for c in range(n_cores)], axis=0)[0]  # [NH]
    const = s.astype(np.float64) @ fcW[NH:, 0].astype(np.float64) + float(fcb[0])
    out = hc + np.float32(const)
    return out.astype(np.float32), res


def kernel(x, adj, W, a, fcW, fcb):
    out, _ = _run_sharded(
        np.asarray(x), np.asarray(adj), np.asarray(W),
        np.asarray(a), np.asarray(fcW), np.asarray(fcb),
    )
    return out


# revision 4
# speedup vs baseline: 1.4692x; 1.1162x over previous
"""GAT layer (nn_GAT_49589692400146) on 8 TRN2 NeuronCores.

Strategy (row-shard over nodes i, SPMD; no cross-core comm):

Math reformulation that removes all full-matrix transcendentals:
  e_ij = lrelu(z1_i + z2_j),  z1 = x@(W a1), z2 = x@(W a2)  (1-D, host fp32)
  p_ij = m_ij * exp(lrelu(t) - 8)
       = m_ij * max(exp(t - 8), exp(0.2 t - 8))          (exp monotone)
       = [m_ij * u1_i] * max(u2_j, (v1_i/u1_i) * v2_j)   (separable!)
  with u = exp(z - 4), v = exp(0.2 z - 4).
  Host precomputes MU[j,i] = m_ij * u1_i (the mask DMA payload, fp16),
  RB[i] = exp(-0.8 z1_i), and per-j columns u2/v2 (fp32).

Device, per core (768 local i-columns, all 6144 j rows), software-pipelined
in groups of 2 j-tiles with P1 leading P2 by LEAD groups:
  P1: Wh chunk [128 nodes, 256] = xT_chunk.T @ W (fp16 matmuls, PSUM),
      evicted to fp16 SBUF by the ACT engine (Copy), + ones column -> who.
  G:  G = (RB_bcast * v2_j) max u2_j     one 4x tensor_scalar per j-tile.
  p:  p = MU * G                          one 2x tensor_tensor per group.
  P2: acc[i] += p_tile^T @ [Wh | 1]       PE, PSUM accumulate [numer|den].
  P3: rec = 1/den; he = elu = min(exp(numer*rec)-1, relu(numer*rec))
      (normalize fused into ACT scale), hc_i = he . fcW_top (DVE accum),
      s = column-sum(he) (ones matmul).
Host: out = concat(hc) + (sum_c s_c) @ fcW_bot + fcb.
"""

import numpy as np

import concourse.bacc as bacc
import concourse.tile as tile
import concourse.mybir as mybir
from concourse import bass_utils

F32 = mybir.dt.float32
F16 = mybir.dt.float16
ALU = mybir.AluOpType
AF = mybir.ActivationFunctionType

NCORES = 8
N_FULL = 6144
NF = 512
NH = 256
ALPHA = 0.2
ESH = 4.0  # per-factor exp shift; total 8 like the reference-safe baseline

GS = 2     # j-tiles per pipeline group
LEAD = 3   # groups P1 runs ahead of P2

_BUILD_CACHE = {}


def _build(NN, R):
    """Build the per-core SPMD module. NN = total nodes (j), R = local i."""
    P = 128
    T = NN // P
    IC = R // P
    KT = NF // P
    assert R % P == 0 and NN % P == 0 and T % GS == 0
    NG = T // GS
    LD = min(LEAD, NG - 1)
    W1 = NH + 1

    nc = bacc.Bacc("TRN2", target_bir_lowering=False, debug=False)

    w_in = nc.dram_tensor("w_in", [P, KT * NH], F16, kind="ExternalInput").ap()
    xtg_in = nc.dram_tensor("xtg_in", [P, KT * NN], F16, kind="ExternalInput").ap()
    mu_in = nc.dram_tensor("mu_in", [P, T * R], F16, kind="ExternalInput").ap()
    rb_in = nc.dram_tensor("rb_in", [1, R], F16, kind="ExternalInput").ap()
    uv_in = nc.dram_tensor("uv_in", [P, 2 * T], F32, kind="ExternalInput").ap()
    fcw_in = nc.dram_tensor("fcw_in", [1, NH], F16, kind="ExternalInput").ap()

    hc_out = nc.dram_tensor("hc_out", [R, 1], F32, kind="ExternalOutput").ap()
    sc_out = nc.dram_tensor("sc_out", [1, NH], F32, kind="ExternalOutput").ap()

    # xtg_in packed group-major: [p, g, k, c] with c in [0, GS*P)
    xtg_r = xtg_in.rearrange("p (g k c) -> p g k c", k=KT, c=GS * P)

    with tile.TileContext(nc) as tc:
        import contextlib

        with contextlib.ExitStack() as ctx:
            pXT = ctx.enter_context(tc.tile_pool(name="pXT", bufs=1))
            pCst = ctx.enter_context(tc.tile_pool(name="pCst", bufs=1))
            pWho = ctx.enter_context(tc.tile_pool(name="pWho", bufs=1))
            pMU = ctx.enter_context(tc.tile_pool(name="pMU", bufs=6))
            pG = ctx.enter_context(tc.tile_pool(name="pG", bufs=6))
            pP = ctx.enter_context(tc.tile_pool(name="pP", bufs=4))
            pS = ctx.enter_context(tc.tile_pool(name="pS", bufs=6))
            psW = ctx.enter_context(tc.tile_pool(name="psW", bufs=2, space="PSUM"))
            psA = ctx.enter_context(tc.tile_pool(name="psA", bufs=1, space="PSUM"))

            # ---- constants / small DMAs first ----
            wch = pCst.tile([P, KT * NH], F16, tag="wch")
            nc.sync.dma_start(wch[:], w_in)
            uv = pCst.tile([P, 2 * T], F32, tag="uv")
            nc.sync.dma_start(uv[:], uv_in)
            rbb = pCst.tile([P, R], F16, tag="rbb")
            nc.gpsimd.dma_start(rbb[:], rb_in.partition_broadcast(P))
            fcwb = pCst.tile([P, NH], F16, tag="fcwb")
            nc.gpsimd.dma_start(fcwb[:], fcw_in.partition_broadcast(P))

            who = pWho.tile([P, T * W1], F16, tag="who")
            who_r = who[:].rearrange("p (t c) -> p t c", c=W1)
            nc.gpsimd.memset(who_r[:, :, NH:W1], 1.0)
            ones_col = pCst.tile([P, 1], F16, tag="ones_col")
            nc.gpsimd.memset(ones_col[:], 1.0)
            hc_sb = pCst.tile([P, IC], F32, tag="hc_sb")
            nc.gpsimd.memset(hc_sb[:], 0.0)

            # xt mega-tile, laid out [p, k*NN + n]
            xtb = pXT.tile([P, KT * NN], F16, tag="xtb")
            xtb_r = xtb[:].rearrange("p (k n) -> p k n", n=NN)

            acc = [
                psA.tile([P, W1], F32, tag=f"acc{i}", name=f"acc{i}")
                for i in range(IC)
            ]

            mug = [None] * NG
            gg = [None] * NG

            def emit_intake(g):
                c0 = g * GS * P
                nc.sync.dma_start(xtb_r[:, :, c0:c0 + GS * P], xtg_r[:, g])
                m = pMU.tile([P, GS * R], F16, tag="mu", name=f"mu{g}")
                nc.sync.dma_start(
                    m[:], mu_in[:, g * GS * R:(g + 1) * GS * R]
                )
                mug[g] = m

            def emit_p1(g):
                for t in range(GS):
                    jt = g * GS + t
                    pc = psW.tile([P, NH], F32, tag="work", name=f"pc{jt}")
                    for k in range(KT):
                        nc.tensor.matmul(
                            pc[:],
                            xtb[:, k * NN + jt * P:k * NN + (jt + 1) * P],
                            wch[:, k * NH:(k + 1) * NH],
                            start=(k == 0),
                            stop=(k == KT - 1),
                        )
                    nc.scalar.activation(
                        who[:, jt * W1:jt * W1 + NH], pc[:], AF.Copy
                    )
                g_t = pG.tile([P, GS * R], F16, tag="gg", name=f"gg{g}")
                for t in range(GS):
                    jt = g * GS + t
                    nc.vector.tensor_scalar(
                        g_t[:, t * R:(t + 1) * R],
                        rbb[:],
                        uv[:, 2 * jt:2 * jt + 1],
                        uv[:, 2 * jt + 1:2 * jt + 2],
                        op0=ALU.mult,
                        op1=ALU.max,
                    )
                gg[g] = g_t

            def emit_p2(g):
                pg = pP.tile([P, GS * R], F16, tag="pg", name=f"pg{g}")
                nc.vector.tensor_tensor(pg[:], mug[g][:], gg[g][:], op=ALU.mult)
                last_g = g == NG - 1
                if last_g:
                    for i in range(IC):
                        for t in range(GS):
                            jt = g * GS + t
                            nc.tensor.matmul(
                                acc[i][:],
                                pg[:, t * R + i * P:t * R + (i + 1) * P],
                                who[:, jt * W1:(jt + 1) * W1],
                                start=(g == 0 and t == 0),
                                stop=(t == GS - 1),
                            )
                else:
                    for t in range(GS):
                        jt = g * GS + t
                        for i in range(IC):
                            nc.tensor.matmul(
                                acc[i][:],
                                pg[:, t * R + i * P:t * R + (i + 1) * P],
                                who[:, jt * W1:(jt + 1) * W1],
                                start=(g == 0 and t == 0),
                                stop=False,
                            )

            for g in range(NG + LD):
                if g < NG:
                    emit_intake(g)
                    emit_p1(g)
                if g >= LD:
                    emit_p2(g - LD)

            # ---- P3: normalize+ELU fused into ACT scale, outputs ----
            sacc = psW.tile([1, NH], F32, tag="work")
            s_sb = pCst.tile([1, NH], F32, tag="s_sb")
            for i in range(IC):
                rec = pS.tile([P, 1], F32, tag="rec")
                nc.vector.reciprocal(rec[:], acc[i][:, NH:W1])
                ex = pS.tile([P, NH], F32, tag="ex")
                nc.scalar.activation(ex[:], acc[i][:, 0:NH], AF.Exp, scale=rec[:])
                rl = pS.tile([P, NH], F32, tag="rl")
                nc.scalar.activation(rl[:], acc[i][:, 0:NH], AF.Relu, scale=rec[:])
                he = pS.tile([P, NH], F16, tag="he")
                nc.vector.scalar_tensor_tensor(
                    he[:], ex[:], -1.0, rl[:], ALU.add, ALU.min
                )
                nc.tensor.matmul(
                    sacc[:], ones_col[:], he[:],
                    start=(i == 0), stop=(i == IC - 1),
                )
                hw = pS.tile([P, NH], F16, tag="hw")
                nc.vector.scalar_tensor_tensor(
                    hw[:], he[:], 1.0, fcwb[:],
                    ALU.mult, ALU.mult, accum_out=hc_sb[:, i:i + 1]
                )

            nc.vector.tensor_copy(s_sb[:], sacc[:])
            nc.sync.dma_start(sc_out, s_sb[:])
            nc.sync.dma_start(
                hc_out.rearrange("(a p) o -> p (a o)", p=P), hc_sb[:]
            )

    nc.compile()
    return nc


def _get_module(NN, R):
    key = (NN, R)
    if key not in _BUILD_CACHE:
        _BUILD_CACHE[key] = _build(NN, R)
    return _BUILD_CACHE[key]


def _make_in_maps(x, adj, W, a, fcW, n_cores=NCORES):
    NN = x.shape[0]
    R = NN // n_cores
    P = 128
    T = NN // P
    KT = NF // P
    NG = T // GS

    x64 = x.astype(np.float64)
    W64 = W.astype(np.float64)
    a64 = a.astype(np.float64)[:, 0]
    z1 = x64 @ (W64 @ a64[:NH])            # [NN]
    z2 = x64 @ (W64 @ a64[NH:])            # [NN]

    u1 = np.exp(z1 - ESH)
    u2 = np.exp(z2 - ESH)
    v2 = np.exp(ALPHA * z2 - ESH)
    rb_full = np.exp(-(1.0 - ALPHA) * z1).astype(np.float16)  # v1/u1

    # x^T packed group-major: [p, g, k, c] -> xT[k*128+p, g*GS*128 + c]
    xT = np.ascontiguousarray(x.T).astype(np.float16)         # [NF, NN]
    xTk = xT.reshape(KT, P, NG, GS * P)                       # [k, p, g, c]
    xtg = np.ascontiguousarray(
        xTk.transpose(1, 2, 0, 3).reshape(P, KT * NN)
    )

    w_t = np.zeros((P, KT * NH), np.float16)
    for k in range(KT):
        w_t[:, k * NH:(k + 1) * NH] = W[k * P:(k + 1) * P, :].astype(np.float16)

    uv = np.zeros((P, 2 * T), np.float32)
    for t in range(T):
        uv[:, 2 * t] = v2[t * P:(t + 1) * P]
        uv[:, 2 * t + 1] = u2[t * P:(t + 1) * P]

    fcw_row = fcW[:NH, 0].astype(np.float16)[None, :]         # [1, NH]

    mask = (adj > 0)                                          # [i, j]
    in_maps = []
    for c in range(n_cores):
        r0, r1 = c * R, (c + 1) * R
        # MU[j, i_local] = mask[i, j] * u1[i]
        mu = (mask[r0:r1, :].T * u1[r0:r1][None, :]).astype(np.float16)  # [NN, R]
        # partition-major layout [p, t*R + i]
        mu_pm = np.ascontiguousarray(
            mu.reshape(T, P, R).transpose(1, 0, 2).reshape(P, T * R)
        )
        in_maps.append({
            "w_in": w_t,
            "xtg_in": xtg,
            "mu_in": mu_pm,
            "rb_in": np.ascontiguousarray(rb_full[r0:r1][None, :]),
            "uv_in": uv,
            "fcw_in": fcw_row,
        })
    return in_maps


def _run_sharded(x, adj, W, a, fcW, fcb, n_cores=NCORES, **run_kwargs):
    NN = x.shape[0]
    R = NN // n_cores
    nc = _get_module(NN, R)
    in_maps = _make_in_maps(x, adj, W, a, fcW, n_cores)

    res = bass_utils.run_bass_kernel_spmd(
        nc, in_maps, core_ids=list(range(n_cores)), **run_kwargs
    )

    hc = np.concatenate([res.results[c]["hc_out"] for c in range(n_cores)], axis=0)
    s = np.sum([res.results[c]["sc_out"] for c in range(n_cores)], axis=0)[0]  # [NH]
    const = s.astype(np.float64) @ fcW[NH:, 0].astype(np.float64) + float(fcb[0])
    out = hc + np.float32(const)
    return out.astype(np.float32), res


def kernel(x, adj, W, a, fcW, fcb):
    out, _ = _run_sharded(
        np.asarray(x), np.asarray(adj), np.asarray(W),
        np.asarray(a), np.asarray(fcW), np.asarray(fcb),
    )
    return out
